# revision 5
# baseline (speedup 1.0000x reference)
"""Multi-head attention block (12 heads, N=2048, C=768) on 8 NeuronCores.

Sharding: core i = (batch b = i//2, head-group g = i%2). Each core computes
attention for 6 heads of one batch plus its slice of the output projection
(row-sharded Wproj); the host sums the two head-group partials per batch.

Per-core dataflow (all matmuls in float32r — full-rate, ~1.6e-4 rel err):
  xT [768,2048] (host-transposed) -> QT/KT [384,2048] (col-major),
  V2 [2048, 6*66] token-major with a ones column per head (66th col = pad).
  Per head h, query-half qh (1024 queries), key-block k (128 keys):
    S^T tile = KT_h[k]^T-block @ QT_h  (PSUM [128,1024])
    expS = exp(S/8)                    (ACT, PSUM->SBUF, one instr)
    U'  += V2_h[k]^T @ expS            (PSUM [66,1024] accumulated over k;
                                        row 64 = softmax denominator via the
                                        ones column)
  U rows are scaled by 1/denom (DMA partition-replicate + DVE mul) into
  UT [384,2048]; out = UT^T-chunks @ Wproj_rows (PSUM-accumulated), DMA out.
"""

import numpy as np
from contextlib import ExitStack

import concourse.bass as bass
import concourse.tile as tile
from concourse import bacc, mybir
from concourse.bass_utils import run_bass_kernel_spmd

N_CORES = 8
C = 768          # model dim
HG = 6           # heads per core
D = 64           # head dim
CHG = HG * D     # 384, per-group qkv width
CC = C // 128    # 6 contraction chunks
MT = CHG // 128  # 3 m-tiles for QT/KT
SCALE = 1.0 / 8.0

F32 = mybir.dt.float32
F32R = mybir.dt.float32r


def build(n_tok: int = 2048):
    NT = n_tok
    KB = NT // 128           # key blocks
    NQ = NT // 512           # 512-wide query chunks
    QH = min(NT, 1024)       # queries per inner pass
    NQH = NT // QH
    QHC = QH // 512          # 512-chunks per query pass

    nc = bacc.Bacc("TRN2", target_bir_lowering=False, debug=False,
                   num_devices=N_CORES)

    xT = nc.dram_tensor("xT", [C, NT], F32R, kind="ExternalInput").ap()
    wq = nc.dram_tensor("wq", [C, CHG], F32R, kind="ExternalInput").ap()
    wk = nc.dram_tensor("wk", [C, CHG], F32R, kind="ExternalInput").ap()
    wv = nc.dram_tensor("wv", [C, CHG], F32R, kind="ExternalInput").ap()
    wp = nc.dram_tensor("wp", [CHG, C], F32R, kind="ExternalInput").ap()
    bqk = nc.dram_tensor("bqk", [128, 2 * MT], F32, kind="ExternalInput").ap()
    bv = nc.dram_tensor("bv", [1, CHG], F32, kind="ExternalInput").ap()
    ones2 = nc.dram_tensor("ones2", [128, 2], F32R, kind="ExternalInput").ap()
    out = nc.dram_tensor("out", [NT, C], F32, kind="ExternalOutput").ap()

    with tile.TileContext(nc) as tc, ExitStack() as ctx:
        wpool = ctx.enter_context(tc.tile_pool(name="w", bufs=1))
        perm = ctx.enter_context(tc.tile_pool(name="perm", bufs=1))
        psum = ctx.enter_context(tc.tile_pool(name="ps", bufs=4, space="PSUM"))

        # ---- persistent SBUF ----
        wq_t = [wpool.tile([128, CHG], F32R, tag=f"wq{c}", name=f"wq{c}") for c in range(CC)]
        wk_t = [wpool.tile([128, CHG], F32R, tag=f"wk{c}", name=f"wk{c}") for c in range(CC)]
        wv_t = [wpool.tile([128, CHG], F32R, tag=f"wv{c}", name=f"wv{c}") for c in range(CC)]
        wp_t = [wpool.tile([128, C], F32R, tag=f"wp{m}", name=f"wp{m}") for m in range(MT)]
        bqk_t = wpool.tile([128, 2 * MT], F32, tag="bqk")
        bv_bc = wpool.tile([128, CHG], F32, tag="bvb")

        QT = [perm.tile([128, NT], F32R, tag=f"qt{m}", name=f"qtt{m}") for m in range(MT)]
        KT = [perm.tile([128, NT], F32R, tag=f"kt{m}", name=f"ktt{m}") for m in range(MT)]
        V2 = [perm.tile([128, HG, 66], F32R, tag=f"v2{t}", name=f"v2t{t}") for t in range(KB)]
        UT = [perm.tile([128, NT], F32R, tag=f"ut{m}", name=f"utt{m}") for m in range(MT)]

        # ---- input DMA ----
        for c in range(CC):
            nc.sync.dma_start(wq_t[c][:], wq[c * 128:(c + 1) * 128, :])
            nc.sync.dma_start(wk_t[c][:], wk[c * 128:(c + 1) * 128, :])
            nc.sync.dma_start(wv_t[c][:], wv[c * 128:(c + 1) * 128, :])
        for m in range(MT):
            nc.sync.dma_start(wp_t[m][:], wp[m * 128:(m + 1) * 128, :])
        nc.sync.dma_start(bqk_t[:], bqk)
        bv_row = wpool.tile([1, CHG], F32, tag="bvr")
        nc.sync.dma_start(bv_row[0:1, :], bv[0:1, :])
        nc.gpsimd.partition_broadcast(bv_bc[:], bv_row[0:1, :])
        for t in range(KB):
            for h in range(HG):
                nc.sync.dma_start(V2[t][:, h, 64:66], ones2[:, 0:2])

        # ---- phase 1: QKV projections (xt pool scoped; space reused later) --
        with tc.tile_pool(name="xt", bufs=1) as xpool:
            xt = []
            for c in range(CC):
                xc = xpool.tile([128, NT], F32R, tag=f"x{c}", name=f"xt{c}")
                nc.sync.dma_start(xc[:], xT[c * 128:(c + 1) * 128, :])
                xt.append(xc)

            def qk_mtile(m):
                for wt, dst, bcol in ((wq_t, QT, m), (wk_t, KT, MT + m)):
                    for n in range(NQ):
                        ps = psum.tile([128, 512], F32, tag="ps", name=f"psqk{m}_{n}")
                        for c in range(CC):
                            nc.tensor.matmul(
                                ps[:], wt[c][:, m * 128:(m + 1) * 128],
                                xt[c][:, n * 512:(n + 1) * 512],
                                start=(c == 0), stop=(c == CC - 1))
                        nc.vector.tensor_scalar_add(
                            dst[m][:, n * 512:(n + 1) * 512], ps[:],
                            bqk_t[:, bcol:bcol + 1])

            qk_mtile(0)
            for t in range(KB):
                ps = psum.tile([128, CHG], F32, tag="ps", name=f"psv{t}")
                for c in range(CC):
                    nc.tensor.matmul(ps[:], xt[c][:, t * 128:(t + 1) * 128],
                                     wv_t[c][:],
                                     start=(c == 0), stop=(c == CC - 1))
                nc.vector.tensor_add(
                    V2[t][:, :, 0:64],
                    ps[:].rearrange("p (h d) -> p h d", h=HG),
                    bv_bc[:].rearrange("p (h d) -> p h d", h=HG))
            for m in range(1, MT):
                qk_mtile(m)

        # ---- phase 2: attention ----
        spool = ctx.enter_context(tc.tile_pool(name="es", bufs=3))
        rpool = ctx.enter_context(tc.tile_pool(name="rb", bufs=2))
        stpool = ctx.enter_context(tc.tile_pool(name="st", bufs=2))
        opool = ctx.enter_context(tc.tile_pool(name="ost", bufs=3))

        for h in range(HG):
            tl, off = h // 2, (h % 2) * 64
            for qh in range(NQH):
                q0 = qh * QH
                psu = psum.tile([128, QH], F32, tag="ps", name=f"psu{h}_{qh}")
                for k in range(KB):
                    pss = psum.tile([128, QH], F32, tag="ps", name=f"pss{h}_{qh}_{k}")
                    for qc in range(QHC):
                        nc.tensor.matmul(
                            pss[:, qc * 512:(qc + 1) * 512],
                            KT[tl][off:off + 64, k * 128:(k + 1) * 128],
                            QT[tl][off:off + 64,
                                   q0 + qc * 512:q0 + (qc + 1) * 512],
                            start=True, stop=True)
                    es = spool.tile([128, QH], F32R, tag="es", name=f"es{h}_{qh}_{k}")
                    nc.scalar.activation(es[:], pss[:],
                                         mybir.ActivationFunctionType.Exp,
                                         scale=SCALE)
                    for qc in range(QHC):
                        nc.tensor.matmul(
                            psu[0:66, qc * 512:(qc + 1) * 512],
                            V2[k][:, h, :],
                            es[:, qc * 512:(qc + 1) * 512],
                            start=(k == 0), stop=(k == KB - 1))
                # denominators -> reciprocal -> replicate -> scale U rows
                rb = rpool.tile([128, QH], F32, tag="rb", name=f"rb{h}_{qh}")
                nc.vector.reciprocal(rb[64:65, :], psu[64:65, :])
                nc.sync.dma_start(rb[0:1, :], rb[64:65, :])
                nc.gpsimd.partition_broadcast(rb[0:64, :], rb[0:1, :])
                if off == 0:
                    nc.vector.tensor_mul(UT[tl][0:64, q0:q0 + QH],
                                         psu[0:64, :], rb[0:64, :])
                else:
                    st = stpool.tile([64, QH], F32R, tag="st", name=f"st{h}_{qh}")
                    nc.vector.tensor_mul(st[0:64, :], psu[0:64, :],
                                         rb[0:64, :])
                    nc.sync.dma_start(UT[tl][64:128, q0:q0 + QH], st[0:64, :])

        # ---- phase 3: output projection ----
        for qb in range(KB):
            psp = psum.tile([128, 1024], F32, tag="ps", name=f"psp{qb}")
            for m in range(MT):
                lhsT = UT[m][:, qb * 128:(qb + 1) * 128]
                nc.tensor.matmul(psp[:, 0:512], lhsT, wp_t[m][:, 0:512],
                                 start=(m == 0), stop=(m == MT - 1))
                nc.tensor.matmul(psp[:, 512:768], lhsT, wp_t[m][:, 512:768],
                                 start=(m == 0), stop=(m == MT - 1))
            ot = opool.tile([128, C], F32, tag="ost", name=f"ot{qb}")
            nc.scalar.copy(ot[:, 0:512], psp[:, 0:512])
            nc.scalar.copy(ot[:, 512:768], psp[:, 512:768])
            nc.sync.dma_start(out[qb * 128:(qb + 1) * 128, :], ot[:])

    nc.compile()
    return nc


_built = {}


def _get_nc(n_tok=2048):
    if n_tok not in _built:
        _built[n_tok] = build(n_tok)
    return _built[n_tok]


def make_in_maps(x, Wqkv, bqkv, Wproj):
    B, NT, _ = x.shape
    x = np.ascontiguousarray(np.asarray(x, dtype=np.float32))
    Wqkv = np.asarray(Wqkv, dtype=np.float32)
    bqkv = np.asarray(bqkv, dtype=np.float32)
    Wproj = np.asarray(Wproj, dtype=np.float32)
    ones2 = np.zeros((128, 2), np.float32)
    ones2[:, 0] = 1.0
    in_maps = []
    for i in range(N_CORES):
        b, g = i // 2, i % 2
        s = g * CHG
        bq = bqkv[s:s + CHG].reshape(MT, 128).T
        bk = bqkv[C + s:C + s + CHG].reshape(MT, 128).T
        in_maps.append({
            "xT": np.ascontiguousarray(x[b].T),
            "wq": np.ascontiguousarray(Wqkv[:, s:s + CHG]),
            "wk": np.ascontiguousarray(Wqkv[:, C + s:C + s + CHG]),
            "wv": np.ascontiguousarray(Wqkv[:, 2 * C + s:2 * C + s + CHG]),
            "wp": np.ascontiguousarray(Wproj[s:s + CHG, :]),
            "bqk": np.ascontiguousarray(np.concatenate([bq, bk], axis=1)),
            "bv": np.ascontiguousarray(bqkv[2 * C + s:2 * C + s + CHG][None, :]),
            "ones2": ones2,
        })
    return in_maps


def gather(results, bproj, B, NT):
    parts = [results[i]["out"] for i in range(N_CORES)]
    out = np.stack([parts[2 * b] + parts[2 * b + 1] for b in range(B)])
    return (out + np.asarray(bproj, np.float32)[None, None, :]).astype(np.float32)


def kernel(x, Wqkv, bqkv, Wproj, bproj, _trace=False):
    x = np.asarray(x)
    B, NT, _ = x.shape
    nc = _get_nc(NT)
    in_maps = make_in_maps(x, Wqkv, bqkv, Wproj)
    res = run_bass_kernel_spmd(nc, in_maps, core_ids=list(range(N_CORES)),
                               trace=_trace)
    out = gather(res.results, bproj, B, NT)
    if _trace:
        return out, res
    return out
